# revision 26
# baseline (speedup 1.0000x reference)
"""Trainium2 Bass kernel for EntropicOTQuantileRegression loss.

Math (per row n of X):
    hx = X @ W1[:DX]; hu = U @ W1[DX:]
    h1 = softplus(hx[n] + hu[m] + b1)          # [m, H] for fixed n
    h2 = softplus(h1 @ W2 + b2)                # [m, H]
    phi[n, m] = h2 @ W3 + b3
    cost[n, m] = Y[n] . U[m]
    psi[n] = EPS * (logsumexp_m((cost - phi)/EPS) - log(M))

Sharding: data-parallel over the n (X/Y row) axis across 8 cores; U and MLP
weights replicated.

This toolchain's cayman ACT tables have no softplus, so softplus is computed
exactly as ln(1 + exp(x)) using only Exp/Ln (both live in the same ACT table
set, natural_log_exp_and_others, so the whole kernel needs one table load).
Layer 1 exploits the rank-1 structure of its pre-activation:
    exp(hx[n] + hu[m] + b1) = exp(hx[n] + b1) * exp(hu[m])
so the Exp pass is amortized (computed once for all n), and per n only a DVE
broadcast-multiply plus one batched Ln(1 + .) ACT pass remain.  Layer 2 is a
[H,H] @ [H,M] bf16 matmul into PSUM, then Exp(. + b2) and a batched Ln(1 + .).

The slackness matrix s = (cost - phi)/EPS is built directly in [n, M] layout
in PSUM by accumulating, for each n, a matmul whose lhsT is a sliding window
over a buffer holding -W3/EPS in one column (so the product lands only in
partition n), plus one f32 matmul for the cost term (lhsT = Y.T/EPS).

Tail: with EPS = 1e-7 the f32 logsumexp degenerates exactly to the row max
(the slackness gaps, ~1e4 in scaled units, dwarf the ~16.6 window below which
exp(s - max) still contributes to a f32 sum; the reference's own f32
logsumexp behaves identically, and even an exact tie would shift psi by only
EPS*ln2 ~ 7e-8).  So the tail is a batched row reduce_max and an affine
combine, psi = EPS*max - b3 - EPS*log(M).
"""

import numpy as np

import concourse.bass as bass
import concourse.tile as tile
from concourse import bacc, mybir
from concourse import bass_utils

N, M, DX, DY, H = 1024, 1024, 64, 16, 128
EPS = 1e-7
SCALE = 1.0 / EPS
N_CORES = 8
NC_ROWS = N // N_CORES  # 128
GRP = 10  # n-rows per batched Ln pass
F32 = mybir.dt.float32
BF16 = mybir.dt.bfloat16

_CACHED_NC = None


def _pin_act_tables_to_combined_set():
    """Make Exp and Ln resolve to the single combined ACT table set.

    The table-load inserter binds each activation to the first table set
    containing its function; Exp's first home (exp_and_others) lacks Ln and
    vice versa, so an Exp/Ln-alternating kernel reloads tables on every
    transition (~1.3us each, 64 times here).  Claiming Exp/Ln exclusively
    for natural_log_exp_and_others (set names/order preserved, so the
    act_func_set_id indexes still match act_info.json) collapses that to
    one load.
    """
    import concourse.bacc as bacc_mod

    orig = bacc_mod.get_activation_tables
    if getattr(bacc_mod, "_act_tables_pinned", False):
        return
    EXP = mybir.ActivationFunctionType.Exp
    LN = mybir.ActivationFunctionType.Ln

    def patched(arch):
        tables = {name: set(fns) for name, fns in orig(arch).items()}
        if "natural_log_exp_and_others" in tables:
            for name, fns in tables.items():
                if name != "natural_log_exp_and_others":
                    fns.discard(EXP)
                    fns.discard(LN)
        return tables

    bacc_mod.get_activation_tables = patched
    bacc_mod._act_tables_pinned = True


def _build():
    _pin_act_tables_to_combined_set()
    from contextlib import ExitStack

    EXP = mybir.ActivationFunctionType.Exp
    LN = mybir.ActivationFunctionType.Ln
    AX = mybir.AxisListType.X

    nc = bacc.Bacc(
        "TRN2", target_bir_lowering=False, debug=False, num_devices=N_CORES
    )

    def din(name, shape):
        return nc.dram_tensor(name, shape, F32, kind="ExternalInput").ap()

    XcT = din("XcT", [DX, NC_ROWS])
    UT = din("UT", [DY, M])
    YsT = din("YsT", [DY, NC_ROWS])  # (1/EPS) * Yc.T
    W1x = din("W1x", [DX, H])
    W1u = din("W1u", [DY, H])
    B1 = din("b1", [H, 1])
    W2 = din("W2", [H, H])
    B2 = din("b2", [H, 1])
    W3s = din("W3s", [H, 1])  # -(1/EPS) * W3
    CB = din("cb", [NC_ROWS, 1])  # -b3 - EPS*log(M), broadcast
    OUT = nc.dram_tensor("out", [NC_ROWS, 1], F32, kind="ExternalOutput").ap()

    with tile.TileContext(nc) as tc, ExitStack() as ctx:
        const = ctx.enter_context(tc.tile_pool(name="const", bufs=1))
        psum_s = ctx.enter_context(tc.tile_pool(name="psum_s", bufs=1, space="PSUM"))
        psum_h = ctx.enter_context(tc.tile_pool(name="psum_h", bufs=3, space="PSUM"))
        e1pool = ctx.enter_context(tc.tile_pool(name="e1p", bufs=2))
        h1pool = ctx.enter_context(tc.tile_pool(name="h1p", bufs=2))
        z2pool = ctx.enter_context(tc.tile_pool(name="z2p", bufs=2))
        h2pool = ctx.enter_context(tc.tile_pool(name="h2p", bufs=2))
        small = ctx.enter_context(tc.tile_pool(name="small", bufs=1))

        # hoist the (single) ACT table load to kernel start: a dependency-free
        # dummy activation makes bacc place the InstLoadActFuncSet here instead
        # of in front of the first real Exp (which waits on DMA + matmul).
        dummy = small.tile([H, 1], F32, tag="dummy")
        nc.vector.memset(dummy[:], 0.0)
        nc.scalar.activation(dummy[:], dummy[:], EXP)

        # input DMAs split across two queues so issue overhead (~0.6us each)
        # doesn't serialize the startup chain; earliest-needed tensors first
        def load(ap, shape, tag, eng):
            t = const.tile(shape, F32, tag=tag)
            eng.dma_start(t[:], ap[:])
            return t

        t_ut = load(UT, [DY, M], "t_ut", nc.sync)
        t_w1u = load(W1u, [DY, H], "t_w1u", nc.gpsimd)
        t_xct = load(XcT, [DX, NC_ROWS], "t_xct", nc.sync)
        t_w1x = load(W1x, [DX, H], "t_w1x", nc.gpsimd)
        t_b1 = load(B1, [H, 1], "t_b1", nc.sync)
        t_w2 = load(W2, [H, H], "t_w2", nc.gpsimd)
        t_yst = load(YsT, [DY, NC_ROWS], "t_yst", nc.sync)
        t_b2 = load(B2, [H, 1], "t_b2", nc.gpsimd)
        t_w3s = load(W3s, [H, 1], "t_w3s", nc.sync)
        t_cb = load(CB, [NC_ROWS, 1], "t_cb", nc.gpsimd)

        # bf16 copies for the TensorEngine-facing tensors
        w2b = const.tile([H, H], BF16, tag="w2b")
        nc.vector.tensor_copy(w2b[:], t_w2[:])
        # sliding-window buffer: column (H-1) holds -W3/EPS, all else zero, so
        # lhsT = w3slide[:, H-1-n : 2H-1-n] puts the product in partition n.
        w3slide = const.tile([H, 2 * H - 1], BF16, tag="w3slide")
        nc.vector.memset(w3slide[:], 0.0)
        nc.vector.tensor_copy(w3slide[:, H - 1 : H], t_w3s[:])

        # ehu = exp(huT) [H, M] first (it gates the broadcast-multiply chain);
        # per-512 halves so each Exp overlaps the other half's matmul.
        # bf16 so the per-n DVE broadcast-multiplies run in the fast mode
        # (the per-partition scalar operand ehxb stays f32).
        p_hu = psum_h.tile([H, M], F32, tag="h2pre")
        ehu = const.tile([H, M], BF16, tag="ehu")
        for b in range(2):
            sl = slice(b * 512, (b + 1) * 512)
            nc.tensor.matmul(p_hu[:, sl], t_w1u[:], t_ut[:, sl], start=True, stop=True)
            nc.scalar.activation(ehu[:, sl], p_hu[:, sl], EXP)

        # ehxb = exp(hxT + b1)  [H, NC_ROWS]
        p_hx = psum_h.tile([H, M], F32, tag="h2pre")
        nc.tensor.matmul(
            p_hx[:, :NC_ROWS], t_w1x[:], t_xct[:], start=True, stop=True
        )
        ehxb = const.tile([H, NC_ROWS], F32, tag="ehxb")
        nc.scalar.activation(ehxb[:], p_hx[:, :NC_ROWS], EXP, bias=t_b1[:])

        # s accumulator in [n, m] layout; first contribution is the cost term
        # (f32 matmul for accuracy: cost dominates the slackness).
        s_all = psum_s.tile([NC_ROWS, M], F32)
        for b in range(2):
            sl = slice(b * 512, (b + 1) * 512)
            nc.tensor.matmul(
                s_all[:, sl],
                t_yst[:],
                t_ut[:, sl],
                start=True,
                stop=False,
                skip_group_check=True,
            )

        # group sizes taper at both ends: small first groups shorten the
        # serial ramp into the ACT pipeline, small last groups shorten the
        # serial drain (last s-matmuls + logsumexp tail).
        sizes = [2, 6] + [GRP] * 11 + [8, 2]
        assert sum(sizes) == NC_ROWS

        def emit_s_mms(h2g, n0, gsz, last_group):
            # accumulate this group's -phi/EPS contributions into s_all
            for b in range(2):
                sl = slice(b * 512, (b + 1) * 512)
                for i in range(gsz):
                    n = n0 + i
                    nc.tensor.matmul(
                        s_all[:, sl],
                        w3slide[:, H - 1 - n : 2 * H - 1 - n],
                        h2g[:, i * M + b * 512 : i * M + (b + 1) * 512],
                        start=False,
                        stop=(last_group and i == gsz - 1),
                        skip_group_check=True,
                    )

        # Software pipeline: each group's s-matmuls are emitted AFTER the next
        # group's W2 matmuls, so PE never head-of-line blocks on the ACT
        # Exp/Ln chain of the current group.
        pending = None  # (h2g, n0, gsz)
        n0 = 0
        for gsz in sizes:
            # stage exp(l1) for gsz rows, then one batched Ln(1+.) pass
            e1g = e1pool.tile([H, gsz * M], BF16, tag="e1g")
            for i in range(gsz):
                n = n0 + i
                nc.vector.tensor_scalar_mul(
                    e1g[:, i * M : (i + 1) * M], ehu[:], ehxb[:, n : n + 1]
                )
            h1g = h1pool.tile([H, gsz * M], BF16, tag="h1g")
            nc.scalar.activation(h1g[:], e1g[:], LN, bias=1.0)

            # layer-2 matmuls into PSUM; DVE stages the pre-activations out to
            # SBUF so both Exp and Ln run as one batched ACT pass per group
            # (and PSUM banks recycle fast enough for PE to stay busy).
            z2g = z2pool.tile([H, gsz * M], BF16, tag="z2g")
            for i in range(gsz):
                h2pre = psum_h.tile([H, M], F32, tag="h2pre")
                for b in range(2):
                    sl = slice(b * 512, (b + 1) * 512)
                    nc.tensor.matmul(
                        h2pre[:, sl],
                        w2b[:],
                        h1g[:, i * M + b * 512 : i * M + (b + 1) * 512],
                        start=True,
                        stop=True,
                    )
                nc.vector.tensor_copy(z2g[:, i * M : (i + 1) * M], h2pre[:])
            if pending is not None:
                emit_s_mms(*pending, last_group=False)
            nc.scalar.activation(z2g[:], z2g[:], EXP, bias=t_b2[:])
            h2g = h2pool.tile([H, gsz * M], BF16, tag="h2g")
            nc.scalar.activation(h2g[:], z2g[:], LN, bias=1.0)
            pending = (h2g, n0, gsz)
            n0 += gsz
        emit_s_mms(*pending, last_group=True)

        # tail: row-logsumexp over the free (m) dim.  In f32 the slackness
        # gaps (min observed ~1.6e-3 * 1/EPS = 1.6e4) dwarf the exp underflow
        # window (~16.6), so sum(exp(s - max)) == 1.0 exactly and the
        # reference's f32 logsumexp equals the row max; even an exact tie
        # would shift psi by only EPS*ln2 ~ 7e-8.  So psi = EPS*max + C.
        # The row-max is computed per 512-block (PSUM bank) so the first
        # reduce overlaps the last group's block-1 matmuls.
        negmax0 = small.tile([NC_ROWS, 1], F32, tag="negmax0")
        negmax1 = small.tile([NC_ROWS, 1], F32, tag="negmax1")
        nc.vector.reduce_max(negmax0[:], s_all[:, :512], axis=AX, negate=True)
        nc.vector.reduce_max(negmax1[:], s_all[:, 512:], axis=AX, negate=True)
        negmax = small.tile([NC_ROWS, 1], F32, tag="negmax")
        nc.vector.tensor_tensor(
            negmax[:], negmax0[:], negmax1[:], op=mybir.AluOpType.min
        )
        res = small.tile([NC_ROWS, 1], F32)
        nc.vector.tensor_scalar(
            res[:],
            negmax[:],
            -EPS,
            t_cb[:],
            op0=mybir.AluOpType.mult,
            op1=mybir.AluOpType.add,
        )
        nc.sync.dma_start(OUT[:], res[:])

    nc.compile()
    return nc


def _get_nc():
    global _CACHED_NC
    if _CACHED_NC is None:
        _CACHED_NC = _build()
    return _CACHED_NC


def _in_maps(X_tensor, U_tensor, Y_tensor, W1, b1, W2, b2, W3, b3):
    f = np.float32
    X_tensor, U_tensor, Y_tensor, W1, b1, W2, b2, W3, b3 = (
        np.asarray(a) for a in (X_tensor, U_tensor, Y_tensor, W1, b1, W2, b2, W3, b3)
    )
    UTv = np.ascontiguousarray(U_tensor.T.astype(f))
    W1xv = np.ascontiguousarray(W1[:DX].astype(f))
    W1uv = np.ascontiguousarray(W1[DX:].astype(f))
    b1v = np.ascontiguousarray(b1.reshape(H, 1).astype(f))
    W2v = np.ascontiguousarray(W2.astype(f))
    b2v = np.ascontiguousarray(b2.reshape(H, 1).astype(f))
    W3sv = np.ascontiguousarray((-SCALE * W3.astype(np.float64)).astype(f)).reshape(
        H, 1
    )
    C = np.float64(-b3[0]) - EPS * np.log(np.float64(M))
    cbv = np.full((NC_ROWS, 1), C, dtype=f)
    maps = []
    for c in range(N_CORES):
        sl = slice(c * NC_ROWS, (c + 1) * NC_ROWS)
        maps.append(
            {
                "XcT": np.ascontiguousarray(X_tensor[sl].T.astype(f)),
                "UT": UTv,
                "YsT": np.ascontiguousarray(
                    (Y_tensor[sl].T.astype(np.float64) * SCALE).astype(f)
                ),
                "W1x": W1xv,
                "W1u": W1uv,
                "b1": b1v,
                "W2": W2v,
                "b2": b2v,
                "W3s": W3sv,
                "cb": cbv,
            }
        )
    return maps


def kernel(X_tensor, U_tensor, Y_tensor, W1, b1, W2, b2, W3, b3, **_ignored):
    import time

    nc = _get_nc()
    maps = _in_maps(X_tensor, U_tensor, Y_tensor, W1, b1, W2, b2, W3, b3)
    last_err = None
    for attempt in range(4):
        try:
            res = bass_utils.run_bass_kernel_spmd(
                nc, maps, core_ids=list(range(N_CORES))
            )
            return np.concatenate(
                [res.results[c]["out"] for c in range(N_CORES)], axis=0
            ).astype(np.float32)
        except Exception as e:  # transient NRT exec-unit faults on first load
            last_err = e
            time.sleep(2.0 * (attempt + 1))
    raise last_err
